# revision 41
# baseline (speedup 1.0000x reference)
"""BatchedDiffPool Trainium2 kernel.

Problem: packed batch of B=32 graphs x 512 nodes, F=128 features, K=64 clusters.
  S_local = softmax(Z @ W + b)                    [N, K]
  Sbd     = block-diag placement of S_local       [N, B*K]
  Zp      = Sbd.T @ Z                             [B*K, F]
  Ap      = Sbd.T @ A @ Sbd                       [B*K, B*K]

Sharding: 8 cores, each owns 4 graphs (2048 rows of A, all 16384 cols).
Per core:
  stage 0: S = softmax(Z@W + b) for ALL nodes (replicated compute, tiny)
  stage 1: for each column-block h (512 cols of graph h):
           Ut[q, c] = sum_p A[p, q] * S_g[p, c]   (A chunks are the matmul
           stationary operand, so no on-chip transposes are ever needed)
  stage 2: ApT[h-block, g-block] = S_h.T @ Ut     (= (S_g.T A_gh S_h).T)
  ZpT_g = Z_g.T @ S_g
A is cast to bf16 on the host (halves DMA, the memory bottleneck) and
pre-tiled per core into [h][partition][row-chunk][col] layout so every DMA is
fully contiguous.  Graphs are permuted per core (local graphs first) so the
single SPMD program uses only static indices; the host unpermutes outputs.
"""

import numpy as np
import ml_dtypes

BF16 = ml_dtypes.bfloat16
FP8 = ml_dtypes.float8_e4m3
A_FP8 = True

# ---- fixed problem config ----
N_TOT = 16384
F = 128
K = 64
B_TOT = 32
N_CORES = 8
GPC = B_TOT // N_CORES      # graphs per core = 4
NPG = N_TOT // B_TOT        # nodes per graph = 512
PCG = NPG // 128            # 128-chunks per graph = 4
N_LOC = GPC * NPG           # local packed nodes = 2048
RC = N_LOC // 128           # local row chunks = 16

_compiled = {}


def _build_nc(b_tot=B_TOT, gpc=GPC, npg=NPG, f=F, k=K, n_cores=N_CORES,
              band_s=False, band_ut=True, a_fp8=False, use_bias=True):
    """Build + compile the per-core SPMD Bass program."""
    from contextlib import ExitStack

    import concourse.bass as bass  # noqa: F401
    import concourse.mybir as mybir
    import concourse.tile as tile
    from concourse import bacc

    pcg = npg // 128
    rc = gpc * pcg              # local row chunks
    n_loc = gpc * npg
    ch_tot = b_tot * pcg        # total S chunks
    assert k == 64 and f == 128 and b_tot % 2 == 0
    bh = b_tot // 2             # h-pairs (two 64-row halves per 128 partitions)

    f32 = mybir.dt.float32
    bf16 = mybir.dt.bfloat16
    adt = mybir.dt.float8e4 if a_fp8 else bf16
    FT = mybir.ActivationFunctionType

    nc = bacc.Bacc("TRN2", target_bir_lowering=False, debug=False,
                   num_devices=n_cores)

    A_t = nc.dram_tensor("A_t", [b_tot, 128, rc * npg], adt, kind="ExternalInput")
    Zt = nc.dram_tensor("Zt", [f, b_tot * npg], bf16, kind="ExternalInput")
    Zl = nc.dram_tensor("Zl", [128, rc * f], bf16, kind="ExternalInput")
    Wb = nc.dram_tensor("Wb", [f, k], bf16, kind="ExternalInput")
    bb = nc.dram_tensor("bb", [1, 4 * k], bf16, kind="ExternalInput")
    S_out = nc.dram_tensor("S_out", [n_loc, k], f32, kind="ExternalOutput")
    ZpT = nc.dram_tensor("ZpT", [f, gpc * k], f32, kind="ExternalOutput")
    # Ap rows for the local graphs, columns in positional-h order
    Ap_out = nc.dram_tensor("Ap_out", [gpc * k, b_tot * k], f32,
                            kind="ExternalOutput")

    with tile.TileContext(nc) as tc, ExitStack() as ctx:
        const = ctx.enter_context(tc.tile_pool(name="const", bufs=1))
        big = ctx.enter_context(tc.tile_pool(name="big", bufs=1))
        apool = ctx.enter_context(tc.tile_pool(name="apool", bufs=6))
        utp = ctx.enter_context(tc.tile_pool(name="utp", bufs=2 * pcg))
        sstat = ctx.enter_context(tc.tile_pool(name="sstat", bufs=4))
        sexp = ctx.enter_context(tc.tile_pool(name="sexp", bufs=6))
        ps_l = ctx.enter_context(tc.tile_pool(name="ps_l", bufs=2, space="PSUM"))
        ps_ut = ctx.enter_context(tc.tile_pool(name="ps_ut", bufs=2, space="PSUM"))
        ps_ap = ctx.enter_context(tc.tile_pool(name="ps_ap", bufs=3, space="PSUM"))

        w_sb = const.tile([f, k], bf16)
        nc.sync.dma_start(out=w_sb, in_=Wb.ap())
        b_sb = const.tile([1, 4 * k], bf16)
        nc.sync.dma_start(out=b_sb, in_=bb.ap())
        ones = const.tile([1, 128], bf16)
        nc.vector.memset(ones, 1.0)

        # Zt in 4 pieces so the softmax stage starts before the full 4 MB lands
        zt_sb = big.tile([f, b_tot * npg], bf16)
        nzt = b_tot * npg // 8
        for j in range(8):
            nc.sync.dma_start(out=zt_sb[:, j * nzt:(j + 1) * nzt],
                              in_=Zt.ap()[:, j * nzt:(j + 1) * nzt])
        zl_sb = big.tile([128, rc * f], bf16)
        nc.sync.dma_start(out=zl_sb, in_=Zl.ap())

        s_bf = big.tile([128, ch_tot * k], bf16)
        if a_fp8:
            s_a = big.tile([128, ch_tot * k], adt, name="s_a")
        else:
            s_a = s_bf
        s_f32 = big.tile([128, rc * k], f32)
        njh = gpc * k // 128
        assert gpc * k == njh * 128
        ap_sb = big.tile([128, njh * b_tot * k], f32)
        zpt_sb = big.tile([f, gpc * k], f32)

        # ---- stage 0: S = softmax(Z @ W + b), 4 node-chunks per PSUM bank ----
        # The per-element has_written bit makes multiple accumulation groups
        # per bank legal: only the first matmul of a bank-round uses
        # start=True (clears the bank's bits); later matmuls overwrite
        # where unwritten / accumulate where written.
        assert ch_tot % 4 == 0

        _sstate = {}

        def softmax_tail(chk, lg_slice):
            # batch the exp-sums of 4 chunks into one tile -> 1 reciprocal op
            j = chk % 4
            if j == 0:
                _sstate["sm"] = sstat.tile([128, 4], f32, name="smb", tag="smb")
                _sstate["rs"] = sstat.tile([128, 4], f32, name="rsb", tag="rsb")
                _sstate["ex"] = []
            ex = sexp.tile([128, k], f32, name="ex", tag="ex")
            nc.scalar.activation(out=ex, in_=lg_slice, func=FT.Exp,
                                 accum_out=_sstate["sm"][:, j:j + 1])
            _sstate["ex"].append((chk, ex))
            if j == 3:
                rs = _sstate["rs"]
                nc.vector.reciprocal(rs, _sstate["sm"])
                for jj, (c2, e2) in enumerate(_sstate["ex"]):
                    nc.vector.tensor_scalar_mul(
                        s_bf[:, c2 * k:(c2 + 1) * k], e2, rs[:, jj:jj + 1])
                    if a_fp8:
                        nc.gpsimd.tensor_copy(s_a[:, c2 * k:(c2 + 1) * k],
                                              s_bf[:, c2 * k:(c2 + 1) * k])
                    if c2 < rc:
                        nc.vector.tensor_scalar_mul(
                            s_f32[:, c2 * k:(c2 + 1) * k], e2, rs[:, jj:jj + 1])

        if not use_bias:
            # fully batched: 4 node-chunks per PSUM bank, one exp per band,
            # per-chunk sums via 3D reduce, stride-0 broadcast normalize
            for bq in range(ch_tot // 4):
                lg = ps_l.tile([128, 4 * k], f32)
                for j in range(4):
                    chk = bq * 4 + j
                    nc.tensor.matmul(lg[:, j * k:(j + 1) * k],
                                     zt_sb[:, chk * 128:(chk + 1) * 128], w_sb,
                                     start=(j == 0), stop=(j == 3),
                                     skip_group_check=True)
                exb = sexp.tile([128, 4 * k], f32)
                nc.scalar.activation(out=exb, in_=lg, func=FT.Exp)
                smb = sstat.tile([128, 4], f32)
                nc.vector.reduce_sum(
                    smb, exb.rearrange("p (j k) -> p j k", k=k),
                    axis=mybir.AxisListType.X)
                rsb = sstat.tile([128, 4], f32)
                nc.vector.reciprocal(rsb, smb)
                rs_b = bass.AP(tensor=rsb.tensor, offset=rsb.offset,
                               ap=[rsb.ap[0], rsb.ap[1], [0, k]])
                nc.vector.tensor_tensor(
                    out=s_bf[:, bq * 4 * k:(bq + 1) * 4 * k].rearrange(
                        "p (j k) -> p j k", k=k),
                    in0=exb.rearrange("p (j k) -> p j k", k=k),
                    in1=rs_b, op=mybir.AluOpType.mult)
                if a_fp8:
                    nc.gpsimd.tensor_copy(
                        s_a[:, bq * 4 * k:(bq + 1) * 4 * k],
                        s_bf[:, bq * 4 * k:(bq + 1) * 4 * k])
                if bq < rc // 4:
                    nc.vector.tensor_tensor(
                        out=s_f32[:, bq * 4 * k:(bq + 1) * 4 * k].rearrange(
                            "p (j k) -> p j k", k=k),
                        in0=exb.rearrange("p (j k) -> p j k", k=k),
                        in1=rs_b, op=mybir.AluOpType.mult)
        else:
            for chk in range(ch_tot):
                lg = ps_l.tile([128, k], f32)
                nc.tensor.matmul(lg, zt_sb[:, chk * 128:(chk + 1) * 128],
                                 w_sb, start=True, stop=False)
                nc.tensor.matmul(lg, ones, b_sb[:, :k], start=False,
                                 stop=True)
                softmax_tail(chk, lg)

        # ---- main loop over column blocks (positional graph index h) ----
        for h in range(b_tot):
            a_sb = apool.tile([128, rc * npg], adt)
            nc.sync.dma_start(out=a_sb, in_=A_t.ap()[h])
            utbs = []
            for qc in range(pcg):
                if band_ut:
                    pu = ps_ut.tile([128, gpc * k], f32)
                    first = True
                    for pc in range(pcg):
                        for g in range(gpc):
                            r = g * pcg + pc
                            nc.tensor.matmul(
                                pu[:, g * k:(g + 1) * k],
                                a_sb[:, r * npg + qc * 128:
                                     r * npg + (qc + 1) * 128],
                                s_a[:, r * k:(r + 1) * k],
                                start=first,
                                stop=(pc == pcg - 1 and g == gpc - 1),
                                skip_group_check=True)
                            first = False
                    utb = utp.tile([128, gpc * k], bf16)
                    if h >= 12 and qc == 3:
                        nc.scalar.copy(utb, pu)
                    else:
                        nc.vector.tensor_copy(utb, pu)
                else:
                    utb = utp.tile([128, gpc * k], bf16)
                    for g in range(gpc):
                        pu = ps_ut.tile([128, k], f32, name="pu_s", tag="pu_s")
                        for pc in range(pcg):
                            r = g * pcg + pc
                            nc.tensor.matmul(
                                pu,
                                a_sb[:, r * npg + qc * 128:
                                     r * npg + (qc + 1) * 128],
                                s_a[:, r * k:(r + 1) * k],
                                start=(pc == 0), stop=(pc == pcg - 1))
                        nc.vector.tensor_copy(utb[:, g * k:(g + 1) * k], pu)
                utbs.append(utb)
            # stage 2: Ap block rows = ut-halves.T @ S_h chunks (FWL-friendly
            # 128-col stationary, untransposed output)
            apbs = [ps_ap.tile([128, k], f32, name="apb", tag="apb")
                    for _ in range(njh)]
            for qc in range(pcg):
                sh = s_bf[:, (h * pcg + qc) * k:(h * pcg + qc + 1) * k]
                for j in range(njh):
                    nc.tensor.matmul(apbs[j], utbs[qc][:, j * 128:(j + 1) * 128],
                                     sh, start=(qc == 0), stop=(qc == pcg - 1))
            for j in range(njh):
                dst = ap_sb[:, j * b_tot * k + h * k: j * b_tot * k + (h + 1) * k]
                if h >= 12:
                    nc.scalar.copy(dst, apbs[j])
                else:
                    nc.vector.tensor_copy(dst, apbs[j])
            # stream finished Ap column groups out during the loop
            if h % 8 == 7:
                h0 = h - 7
                for j in range(njh):
                    nc.sync.dma_start(
                        out=Ap_out.ap().rearrange("(j p) c -> p j c", p=128)
                        [:, j, h0 * k:(h + 1) * k],
                        in_=ap_sb[:, j * b_tot * k + h0 * k:
                                  j * b_tot * k + (h + 1) * k])

        # ---- ZpT: pooled features (transposed), local graphs ----
        if band_ut:
            zp = ps_l.tile([f, gpc * k], f32, name="zp", tag="zp", bufs=1)
            first = True
            for pc in range(pcg):
                for g in range(gpc):
                    r = g * pcg + pc
                    nc.tensor.matmul(zp[:, g * k:(g + 1) * k],
                                     zl_sb[:, r * f:(r + 1) * f],
                                     s_bf[:, r * k:(r + 1) * k],
                                     start=first,
                                     stop=(pc == pcg - 1 and g == gpc - 1),
                                     skip_group_check=True)
                    first = False
            nc.vector.tensor_copy(zpt_sb, zp)
        else:
            for g in range(gpc):
                zp = ps_l.tile([f, k], f32, name="zp_s", tag="zp_s", bufs=1)
                for pc in range(pcg):
                    r = g * pcg + pc
                    nc.tensor.matmul(zp, zl_sb[:, r * f:(r + 1) * f],
                                     s_bf[:, r * k:(r + 1) * k],
                                     start=(pc == 0), stop=(pc == pcg - 1))
                nc.vector.tensor_copy(zpt_sb[:, g * k:(g + 1) * k], zp)

        # ---- output DMAs ----
        nc.sync.dma_start(
            out=S_out.ap().rearrange("(n p) k -> p n k", p=128),
            in_=s_f32.rearrange("p (n k) -> p n k", k=k))
        nc.sync.dma_start(out=ZpT.ap(), in_=zpt_sb)
        if b_tot % 8 != 0:
            for j in range(njh):
                nc.sync.dma_start(
                    out=Ap_out.ap().rearrange("(j p) c -> p j c", p=128)[:, j, :],
                    in_=ap_sb[:, j * b_tot * k:(j + 1) * b_tot * k])

    nc.compile()
    return nc


def _perm(core, b_tot=B_TOT, gpc=GPC):
    g0 = core * gpc
    return list(range(g0, g0 + gpc)) + \
        [g for g in range(b_tot) if not (g0 <= g < g0 + gpc)]


def _prep_core(Zbf, Abf, Wbf, bbf, core, b_tot=B_TOT, gpc=GPC, npg=NPG, f=F,
               k=K):
    n_loc = gpc * npg
    rc = n_loc // 128
    perm = _perm(core, b_tot, gpc)
    rows = slice(core * n_loc, (core + 1) * n_loc)

    Ar = Abf[rows].reshape(rc, 128, b_tot, npg)      # [rc, p, h_global, q]
    Ar = Ar[:, :, perm, :]                           # positional h
    A_t = np.ascontiguousarray(Ar.transpose(2, 1, 0, 3)).reshape(
        b_tot, 128, rc * npg)

    pnodes = np.concatenate([np.arange(g * npg, (g + 1) * npg) for g in perm])
    Zt = np.ascontiguousarray(Zbf[pnodes].T)         # [f, b_tot*npg]
    Zl = np.ascontiguousarray(
        Zbf[rows].reshape(rc, 128, f).transpose(1, 0, 2)).reshape(128, rc * f)
    return {"A_t": A_t, "Zt": Zt, "Zl": Zl, "Wb": Wbf,
            "bb": np.tile(bbf, 4).reshape(1, 4 * k)}


def _assemble(results, n_nodes, b_tot=B_TOT, gpc=GPC, npg=NPG, f=F, k=K,
              n_cores=N_CORES):
    n_tot = b_tot * npg
    S_local = np.ascontiguousarray(
        np.vstack([results[c]["S_out"] for c in range(n_cores)]),
        dtype=np.float32)

    Zp = np.vstack([
        results[c]["ZpT"].reshape(f, gpc, k).transpose(1, 2, 0).reshape(gpc * k, f)
        for c in range(n_cores)]).astype(np.float32, copy=False)

    ap_rows = []
    for c in range(n_cores):
        X = results[c]["Ap_out"].reshape(gpc * k, b_tot, k)  # [row, hpos, d]
        Y = np.empty_like(X)
        Y[:, _perm(c, b_tot, gpc), :] = X                    # unpermute h
        ap_rows.append(Y.reshape(gpc * k, b_tot * k))
    Ap = np.vstack(ap_rows).astype(np.float32, copy=False)

    seg_ids = np.repeat(np.arange(b_tot, dtype=np.int32), k)
    gids = np.repeat(np.arange(b_tot), n_nodes)
    Sbd = np.zeros((n_tot, b_tot * k), np.float32)
    Sbd[np.arange(n_tot)[:, None], gids[:, None] * k + np.arange(k)[None, :]] = \
        S_local
    return Zp, Ap, seg_ids, S_local, Sbd


def kernel(Z, A, n_nodes, W, b):
    from concourse.bass_utils import run_bass_kernel_spmd

    Z = np.asarray(Z, dtype=np.float32)
    A = np.asarray(A, dtype=np.float32)
    W = np.asarray(W, dtype=np.float32)
    b = np.asarray(b, dtype=np.float32)
    n_nodes = np.asarray(n_nodes)

    a_fp8 = A_FP8
    use_bias = bool(np.any(b != 0.0))
    key = (a_fp8, use_bias)
    if key not in _compiled:
        _compiled[key] = _build_nc(a_fp8=a_fp8, use_bias=use_bias)
    nc = _compiled[key]

    Zbf = Z.astype(BF16)
    Abf = A.astype(FP8 if a_fp8 else BF16)
    Wbf = W.astype(BF16)
    bbf = b.astype(BF16)
    in_maps = [_prep_core(Zbf, Abf, Wbf, bbf, c) for c in range(N_CORES)]
    res = run_bass_kernel_spmd(nc, in_maps, core_ids=list(range(N_CORES)))
    return _assemble(res.results, n_nodes)


# revision 46
# speedup vs baseline: 1.0401x; 1.0401x over previous
"""BatchedDiffPool Trainium2 kernel.

Problem: packed batch of B=32 graphs x 512 nodes, F=128 features, K=64 clusters.
  S_local = softmax(Z @ W + b)                    [N, K]
  Sbd     = block-diag placement of S_local       [N, B*K]
  Zp      = Sbd.T @ Z                             [B*K, F]
  Ap      = Sbd.T @ A @ Sbd                       [B*K, B*K]

Sharding: 8 cores, each owns 4 graphs (2048 rows of A, all 16384 cols).
Per core:
  stage 0: S = softmax(Z@W + b) for ALL nodes (replicated compute, tiny)
  stage 1: for each column-block h (512 cols of graph h):
           Ut[q, c] = sum_p A[p, q] * S_g[p, c]   (A chunks are the matmul
           stationary operand, so no on-chip transposes are ever needed)
  stage 2: ApT[h-block, g-block] = S_h.T @ Ut     (= (S_g.T A_gh S_h).T)
  ZpT_g = Z_g.T @ S_g
A is cast to bf16 on the host (halves DMA, the memory bottleneck) and
pre-tiled per core into [h][partition][row-chunk][col] layout so every DMA is
fully contiguous.  Graphs are permuted per core (local graphs first) so the
single SPMD program uses only static indices; the host unpermutes outputs.
"""

import numpy as np
import ml_dtypes

BF16 = ml_dtypes.bfloat16
FP8 = ml_dtypes.float8_e4m3
A_FP8 = True

# ---- fixed problem config ----
N_TOT = 16384
F = 128
K = 64
B_TOT = 32
N_CORES = 8
GPC = B_TOT // N_CORES      # graphs per core = 4
NPG = N_TOT // B_TOT        # nodes per graph = 512
PCG = NPG // 128            # 128-chunks per graph = 4
N_LOC = GPC * NPG           # local packed nodes = 2048
RC = N_LOC // 128           # local row chunks = 16

_compiled = {}


def _build_nc(b_tot=B_TOT, gpc=GPC, npg=NPG, f=F, k=K, n_cores=N_CORES,
              band_s=False, band_ut=True, a_fp8=False, use_bias=True):
    """Build + compile the per-core SPMD Bass program."""
    from contextlib import ExitStack

    import concourse.bass as bass  # noqa: F401
    import concourse.mybir as mybir
    import concourse.tile as tile
    from concourse import bacc

    pcg = npg // 128
    rc = gpc * pcg              # local row chunks
    n_loc = gpc * npg
    ch_tot = b_tot * pcg        # total S chunks
    assert k == 64 and f == 128 and b_tot % 2 == 0
    bh = b_tot // 2             # h-pairs (two 64-row halves per 128 partitions)

    f32 = mybir.dt.float32
    bf16 = mybir.dt.bfloat16
    adt = mybir.dt.float8e4 if a_fp8 else bf16
    FT = mybir.ActivationFunctionType

    nc = bacc.Bacc("TRN2", target_bir_lowering=False, debug=False,
                   num_devices=n_cores)

    A_t = nc.dram_tensor("A_t", [b_tot, 128, rc * npg], adt, kind="ExternalInput")
    Zt = nc.dram_tensor("Zt", [f, b_tot * npg], bf16, kind="ExternalInput")
    Zl = nc.dram_tensor("Zl", [128, rc * f], bf16, kind="ExternalInput")
    Wb = nc.dram_tensor("Wb", [f, k], bf16, kind="ExternalInput")
    bb = nc.dram_tensor("bb", [1, 4 * k], bf16, kind="ExternalInput")
    S_out = nc.dram_tensor("S_out", [n_loc, k], f32, kind="ExternalOutput")
    ZpT = nc.dram_tensor("ZpT", [f, gpc * k], f32, kind="ExternalOutput")
    # Ap rows for the local graphs, columns in positional-h order
    Ap_out = nc.dram_tensor("Ap_out", [gpc * k, b_tot * k], f32,
                            kind="ExternalOutput")

    with tile.TileContext(nc) as tc, ExitStack() as ctx:
        const = ctx.enter_context(tc.tile_pool(name="const", bufs=1))
        big = ctx.enter_context(tc.tile_pool(name="big", bufs=1))
        apool = ctx.enter_context(tc.tile_pool(name="apool", bufs=8))
        utp = ctx.enter_context(tc.tile_pool(name="utp", bufs=2 * pcg))
        sstat = ctx.enter_context(tc.tile_pool(name="sstat", bufs=4))
        sexp = ctx.enter_context(tc.tile_pool(name="sexp", bufs=6))
        ps_l = ctx.enter_context(tc.tile_pool(name="ps_l", bufs=2, space="PSUM"))
        ps_ut = ctx.enter_context(tc.tile_pool(name="ps_ut", bufs=2, space="PSUM"))
        ps_ap = ctx.enter_context(tc.tile_pool(name="ps_ap", bufs=3, space="PSUM"))

        w_sb = const.tile([f, k], bf16)
        nc.sync.dma_start(out=w_sb, in_=Wb.ap())
        b_sb = const.tile([1, 4 * k], bf16)
        nc.sync.dma_start(out=b_sb, in_=bb.ap())
        ones = const.tile([1, 128], bf16)
        nc.vector.memset(ones, 1.0)

        # Zt in 4 pieces so the softmax stage starts before the full 4 MB lands
        zt_sb = big.tile([f, b_tot * npg], bf16)
        nzt = b_tot * npg // 8
        for j in range(8):
            nc.sync.dma_start(out=zt_sb[:, j * nzt:(j + 1) * nzt],
                              in_=Zt.ap()[:, j * nzt:(j + 1) * nzt])
        zl_sb = big.tile([128, rc * f], bf16)
        nc.sync.dma_start(out=zl_sb, in_=Zl.ap())

        s_bf = big.tile([128, ch_tot * k], bf16)
        if a_fp8:
            # fp8 shadow of S needed only for the LOCAL chunks (stage-1 rhs)
            s_a = big.tile([128, rc * k], adt, name="s_a")
        else:
            s_a = s_bf
        s_f32 = big.tile([128, rc * k], f32)
        njh = gpc * k // 128
        assert gpc * k == njh * 128
        ap_sb = big.tile([128, njh * b_tot * k], f32)
        zpt_sb = big.tile([f, gpc * k], f32)

        # ---- stage 0: S = softmax(Z @ W + b), 4 node-chunks per PSUM bank ----
        # The per-element has_written bit makes multiple accumulation groups
        # per bank legal: only the first matmul of a bank-round uses
        # start=True (clears the bank's bits); later matmuls overwrite
        # where unwritten / accumulate where written.
        assert ch_tot % 4 == 0

        _sstate = {}

        def softmax_tail(chk, lg_slice):
            # batch the exp-sums of 4 chunks into one tile -> 1 reciprocal op
            j = chk % 4
            if j == 0:
                _sstate["sm"] = sstat.tile([128, 4], f32, name="smb", tag="smb")
                _sstate["rs"] = sstat.tile([128, 4], f32, name="rsb", tag="rsb")
                _sstate["ex"] = []
            ex = sexp.tile([128, k], f32, name="ex", tag="ex")
            nc.scalar.activation(out=ex, in_=lg_slice, func=FT.Exp,
                                 accum_out=_sstate["sm"][:, j:j + 1])
            _sstate["ex"].append((chk, ex))
            if j == 3:
                rs = _sstate["rs"]
                nc.vector.reciprocal(rs, _sstate["sm"])
                for jj, (c2, e2) in enumerate(_sstate["ex"]):
                    nc.vector.tensor_scalar_mul(
                        s_bf[:, c2 * k:(c2 + 1) * k], e2, rs[:, jj:jj + 1])
                    if a_fp8 and c2 < rc:
                        nc.gpsimd.tensor_copy(s_a[:, c2 * k:(c2 + 1) * k],
                                              s_bf[:, c2 * k:(c2 + 1) * k])
                    if c2 < rc:
                        nc.vector.tensor_scalar_mul(
                            s_f32[:, c2 * k:(c2 + 1) * k], e2, rs[:, jj:jj + 1])

        if not use_bias:
            # fully batched: 4 node-chunks per PSUM bank, one exp per band,
            # per-chunk sums via 3D reduce, stride-0 broadcast normalize
            for bq in range(ch_tot // 4):
                lg = ps_l.tile([128, 4 * k], f32)
                for j in range(4):
                    chk = bq * 4 + j
                    nc.tensor.matmul(lg[:, j * k:(j + 1) * k],
                                     zt_sb[:, chk * 128:(chk + 1) * 128], w_sb,
                                     start=(j == 0), stop=(j == 3),
                                     skip_group_check=True)
                exb = sexp.tile([128, 4 * k], f32)
                nc.scalar.activation(out=exb, in_=lg, func=FT.Exp)
                smb = sstat.tile([128, 4], f32)
                nc.vector.reduce_sum(
                    smb, exb.rearrange("p (j k) -> p j k", k=k),
                    axis=mybir.AxisListType.X)
                rsb = sstat.tile([128, 4], f32)
                nc.vector.reciprocal(rsb, smb)
                rs_b = bass.AP(tensor=rsb.tensor, offset=rsb.offset,
                               ap=[rsb.ap[0], rsb.ap[1], [0, k]])
                nc.vector.tensor_tensor(
                    out=s_bf[:, bq * 4 * k:(bq + 1) * 4 * k].rearrange(
                        "p (j k) -> p j k", k=k),
                    in0=exb.rearrange("p (j k) -> p j k", k=k),
                    in1=rs_b, op=mybir.AluOpType.mult)
                if a_fp8 and bq < rc // 4:
                    nc.gpsimd.tensor_copy(
                        s_a[:, bq * 4 * k:(bq + 1) * 4 * k],
                        s_bf[:, bq * 4 * k:(bq + 1) * 4 * k])
                if bq < rc // 4:
                    nc.vector.tensor_tensor(
                        out=s_f32[:, bq * 4 * k:(bq + 1) * 4 * k].rearrange(
                            "p (j k) -> p j k", k=k),
                        in0=exb.rearrange("p (j k) -> p j k", k=k),
                        in1=rs_b, op=mybir.AluOpType.mult)
        else:
            for chk in range(ch_tot):
                lg = ps_l.tile([128, k], f32)
                nc.tensor.matmul(lg, zt_sb[:, chk * 128:(chk + 1) * 128],
                                 w_sb, start=True, stop=False)
                nc.tensor.matmul(lg, ones, b_sb[:, :k], start=False,
                                 stop=True)
                softmax_tail(chk, lg)

        # ---- main loop over column blocks (positional graph index h) ----
        for h in range(b_tot):
            a_sb = apool.tile([128, rc * npg], adt)
            nc.sync.dma_start(out=a_sb, in_=A_t.ap()[h])
            utbs = []
            for qc in range(pcg):
                if band_ut:
                    pu = ps_ut.tile([128, gpc * k], f32)
                    first = True
                    for pc in range(pcg):
                        for g in range(gpc):
                            r = g * pcg + pc
                            nc.tensor.matmul(
                                pu[:, g * k:(g + 1) * k],
                                a_sb[:, r * npg + qc * 128:
                                     r * npg + (qc + 1) * 128],
                                s_a[:, r * k:(r + 1) * k],
                                start=first,
                                stop=(pc == pcg - 1 and g == gpc - 1),
                                skip_group_check=True)
                            first = False
                    utb = utp.tile([128, gpc * k], bf16)
                    if h >= 12 and qc == 3:
                        nc.scalar.copy(utb, pu)
                    else:
                        nc.vector.tensor_copy(utb, pu)
                else:
                    utb = utp.tile([128, gpc * k], bf16)
                    for g in range(gpc):
                        pu = ps_ut.tile([128, k], f32, name="pu_s", tag="pu_s")
                        for pc in range(pcg):
                            r = g * pcg + pc
                            nc.tensor.matmul(
                                pu,
                                a_sb[:, r * npg + qc * 128:
                                     r * npg + (qc + 1) * 128],
                                s_a[:, r * k:(r + 1) * k],
                                start=(pc == 0), stop=(pc == pcg - 1))
                        nc.vector.tensor_copy(utb[:, g * k:(g + 1) * k], pu)
                utbs.append(utb)
            # stage 2: Ap block rows = ut-halves.T @ S_h chunks (FWL-friendly
            # 128-col stationary, untransposed output)
            apbs = [ps_ap.tile([128, k], f32, name="apb", tag="apb")
                    for _ in range(njh)]
            for qc in range(pcg):
                sh = s_bf[:, (h * pcg + qc) * k:(h * pcg + qc + 1) * k]
                for j in range(njh):
                    nc.tensor.matmul(apbs[j], utbs[qc][:, j * 128:(j + 1) * 128],
                                     sh, start=(qc == 0), stop=(qc == pcg - 1))
            for j in range(njh):
                dst = ap_sb[:, j * b_tot * k + h * k: j * b_tot * k + (h + 1) * k]
                if h >= 12:
                    nc.scalar.copy(dst, apbs[j])
                else:
                    nc.vector.tensor_copy(dst, apbs[j])
            # stream finished Ap column groups out during the loop
            if h % 8 == 7:
                h0 = h - 7
                for j in range(njh):
                    nc.scalar.dma_start(
                        out=Ap_out.ap().rearrange("(j p) c -> p j c", p=128)
                        [:, j, h0 * k:(h + 1) * k],
                        in_=ap_sb[:, j * b_tot * k + h0 * k:
                                  j * b_tot * k + (h + 1) * k])

        # ---- ZpT: pooled features (transposed), local graphs ----
        if band_ut:
            zp = ps_l.tile([f, gpc * k], f32, name="zp", tag="zp", bufs=1)
            first = True
            for pc in range(pcg):
                for g in range(gpc):
                    r = g * pcg + pc
                    nc.tensor.matmul(zp[:, g * k:(g + 1) * k],
                                     zl_sb[:, r * f:(r + 1) * f],
                                     s_bf[:, r * k:(r + 1) * k],
                                     start=first,
                                     stop=(pc == pcg - 1 and g == gpc - 1),
                                     skip_group_check=True)
                    first = False
            nc.vector.tensor_copy(zpt_sb, zp)
        else:
            for g in range(gpc):
                zp = ps_l.tile([f, k], f32, name="zp_s", tag="zp_s", bufs=1)
                for pc in range(pcg):
                    r = g * pcg + pc
                    nc.tensor.matmul(zp, zl_sb[:, r * f:(r + 1) * f],
                                     s_bf[:, r * k:(r + 1) * k],
                                     start=(pc == 0), stop=(pc == pcg - 1))
                nc.vector.tensor_copy(zpt_sb[:, g * k:(g + 1) * k], zp)

        # ---- output DMAs ----
        nc.scalar.dma_start(
            out=S_out.ap().rearrange("(n p) k -> p n k", p=128),
            in_=s_f32.rearrange("p (n k) -> p n k", k=k))
        nc.scalar.dma_start(out=ZpT.ap(), in_=zpt_sb)
        if b_tot % 8 != 0:
            for j in range(njh):
                nc.scalar.dma_start(
                    out=Ap_out.ap().rearrange("(j p) c -> p j c", p=128)[:, j, :],
                    in_=ap_sb[:, j * b_tot * k:(j + 1) * b_tot * k])

    nc.compile()
    return nc


def _perm(core, b_tot=B_TOT, gpc=GPC):
    g0 = core * gpc
    return list(range(g0, g0 + gpc)) + \
        [g for g in range(b_tot) if not (g0 <= g < g0 + gpc)]


def _prep_core(Zbf, Abf, Wbf, bbf, core, b_tot=B_TOT, gpc=GPC, npg=NPG, f=F,
               k=K):
    n_loc = gpc * npg
    rc = n_loc // 128
    perm = _perm(core, b_tot, gpc)
    rows = slice(core * n_loc, (core + 1) * n_loc)

    Ar = Abf[rows].reshape(rc, 128, b_tot, npg)      # [rc, p, h_global, q]
    Ar = Ar[:, :, perm, :]                           # positional h
    A_t = np.ascontiguousarray(Ar.transpose(2, 1, 0, 3)).reshape(
        b_tot, 128, rc * npg)

    pnodes = np.concatenate([np.arange(g * npg, (g + 1) * npg) for g in perm])
    Zt = np.ascontiguousarray(Zbf[pnodes].T)         # [f, b_tot*npg]
    Zl = np.ascontiguousarray(
        Zbf[rows].reshape(rc, 128, f).transpose(1, 0, 2)).reshape(128, rc * f)
    return {"A_t": A_t, "Zt": Zt, "Zl": Zl, "Wb": Wbf,
            "bb": np.tile(bbf, 4).reshape(1, 4 * k)}


def _assemble(results, n_nodes, b_tot=B_TOT, gpc=GPC, npg=NPG, f=F, k=K,
              n_cores=N_CORES):
    n_tot = b_tot * npg
    S_local = np.ascontiguousarray(
        np.vstack([results[c]["S_out"] for c in range(n_cores)]),
        dtype=np.float32)

    Zp = np.vstack([
        results[c]["ZpT"].reshape(f, gpc, k).transpose(1, 2, 0).reshape(gpc * k, f)
        for c in range(n_cores)]).astype(np.float32, copy=False)

    ap_rows = []
    for c in range(n_cores):
        X = results[c]["Ap_out"].reshape(gpc * k, b_tot, k)  # [row, hpos, d]
        Y = np.empty_like(X)
        Y[:, _perm(c, b_tot, gpc), :] = X                    # unpermute h
        ap_rows.append(Y.reshape(gpc * k, b_tot * k))
    Ap = np.vstack(ap_rows).astype(np.float32, copy=False)

    seg_ids = np.repeat(np.arange(b_tot, dtype=np.int32), k)
    gids = np.repeat(np.arange(b_tot), n_nodes)
    Sbd = np.zeros((n_tot, b_tot * k), np.float32)
    Sbd[np.arange(n_tot)[:, None], gids[:, None] * k + np.arange(k)[None, :]] = \
        S_local
    return Zp, Ap, seg_ids, S_local, Sbd


def kernel(Z, A, n_nodes, W, b):
    from concourse.bass_utils import run_bass_kernel_spmd

    Z = np.asarray(Z, dtype=np.float32)
    A = np.asarray(A, dtype=np.float32)
    W = np.asarray(W, dtype=np.float32)
    b = np.asarray(b, dtype=np.float32)
    n_nodes = np.asarray(n_nodes)

    a_fp8 = A_FP8
    use_bias = bool(np.any(b != 0.0))
    key = (a_fp8, use_bias)
    if key not in _compiled:
        _compiled[key] = _build_nc(a_fp8=a_fp8, use_bias=use_bias)
    nc = _compiled[key]

    Zbf = Z.astype(BF16)
    Abf = A.astype(FP8 if a_fp8 else BF16)
    Wbf = W.astype(BF16)
    bbf = b.astype(BF16)
    in_maps = [_prep_core(Zbf, Abf, Wbf, bbf, c) for c in range(N_CORES)]
    res = run_bass_kernel_spmd(nc, in_maps, core_ids=list(range(N_CORES)))
    return _assemble(res.results, n_nodes)


# revision 48
# speedup vs baseline: 1.1592x; 1.1145x over previous
"""BatchedDiffPool Trainium2 kernel.

Problem: packed batch of B=32 graphs x 512 nodes, F=128 features, K=64 clusters.
  S_local = softmax(Z @ W + b)                    [N, K]
  Sbd     = block-diag placement of S_local       [N, B*K]
  Zp      = Sbd.T @ Z                             [B*K, F]
  Ap      = Sbd.T @ A @ Sbd                       [B*K, B*K]

Sharding: 8 cores, each owns 4 graphs (2048 rows of A, all 16384 cols).
Per core:
  stage 0: S = softmax(Z@W + b) for ALL nodes (replicated compute, tiny)
  stage 1: for each column-block h (512 cols of graph h):
           Ut[q, c] = sum_p A[p, q] * S_g[p, c]   (A chunks are the matmul
           stationary operand, so no on-chip transposes are ever needed)
  stage 2: ApT[h-block, g-block] = S_h.T @ Ut     (= (S_g.T A_gh S_h).T)
  ZpT_g = Z_g.T @ S_g
A is cast to bf16 on the host (halves DMA, the memory bottleneck) and
pre-tiled per core into [h][partition][row-chunk][col] layout so every DMA is
fully contiguous.  Graphs are permuted per core (local graphs first) so the
single SPMD program uses only static indices; the host unpermutes outputs.
"""

import numpy as np
import ml_dtypes

BF16 = ml_dtypes.bfloat16
FP8 = ml_dtypes.float8_e4m3
A_FP8 = True

# ---- fixed problem config ----
N_TOT = 16384
F = 128
K = 64
B_TOT = 32
N_CORES = 8
GPC = B_TOT // N_CORES      # graphs per core = 4
NPG = N_TOT // B_TOT        # nodes per graph = 512
PCG = NPG // 128            # 128-chunks per graph = 4
N_LOC = GPC * NPG           # local packed nodes = 2048
RC = N_LOC // 128           # local row chunks = 16

_compiled = {}


def _build_nc(b_tot=B_TOT, gpc=GPC, npg=NPG, f=F, k=K, n_cores=N_CORES,
              band_s=False, band_ut=True, a_fp8=False, use_bias=True):
    """Build + compile the per-core SPMD Bass program."""
    from contextlib import ExitStack

    import concourse.bass as bass  # noqa: F401
    import concourse.mybir as mybir
    import concourse.tile as tile
    from concourse import bacc

    pcg = npg // 128
    rc = gpc * pcg              # local row chunks
    n_loc = gpc * npg
    ch_tot = b_tot * pcg        # total S chunks
    assert k == 64 and f == 128 and b_tot % 2 == 0
    bh = b_tot // 2             # h-pairs (two 64-row halves per 128 partitions)

    f32 = mybir.dt.float32
    bf16 = mybir.dt.bfloat16
    adt = mybir.dt.float8e4 if a_fp8 else bf16
    FT = mybir.ActivationFunctionType

    nc = bacc.Bacc("TRN2", target_bir_lowering=False, debug=False,
                   num_devices=n_cores)

    A_t = nc.dram_tensor("A_t", [b_tot, 128, rc * npg], adt, kind="ExternalInput")
    Zt = nc.dram_tensor("Zt", [f, b_tot * npg], bf16, kind="ExternalInput")
    Zl = nc.dram_tensor("Zl", [128, rc * f], bf16, kind="ExternalInput")
    Wb = nc.dram_tensor("Wb", [f, k], bf16, kind="ExternalInput")
    bb = nc.dram_tensor("bb", [1, 4 * k], bf16, kind="ExternalInput")
    S_out = nc.dram_tensor("S_out", [n_loc, k], f32, kind="ExternalOutput")
    ZpT = nc.dram_tensor("ZpT", [f, gpc * k], f32, kind="ExternalOutput")
    # Ap rows for the local graphs, columns in positional-h order
    Ap_out = nc.dram_tensor("Ap_out", [gpc * k, b_tot * k], f32,
                            kind="ExternalOutput")

    with tile.TileContext(nc) as tc, ExitStack() as ctx:
        const = ctx.enter_context(tc.tile_pool(name="const", bufs=1))
        big = ctx.enter_context(tc.tile_pool(name="big", bufs=1))
        apool = ctx.enter_context(tc.tile_pool(name="apool", bufs=8))
        utp = ctx.enter_context(tc.tile_pool(name="utp", bufs=2 * pcg))
        sstat = ctx.enter_context(tc.tile_pool(name="sstat", bufs=4))
        sexp = ctx.enter_context(tc.tile_pool(name="sexp", bufs=6))
        ps_l = ctx.enter_context(tc.tile_pool(name="ps_l", bufs=2, space="PSUM"))
        ps_ut = ctx.enter_context(tc.tile_pool(name="ps_ut", bufs=2, space="PSUM"))
        ps_ap = ctx.enter_context(tc.tile_pool(name="ps_ap", bufs=3, space="PSUM"))

        w_sb = const.tile([f, k], bf16)
        nc.sync.dma_start(out=w_sb, in_=Wb.ap())
        b_sb = const.tile([1, 4 * k], bf16)
        nc.sync.dma_start(out=b_sb, in_=bb.ap())
        ones = const.tile([1, 128], bf16)
        nc.vector.memset(ones, 1.0)

        # Zt in 4 pieces so the softmax stage starts before the full 4 MB lands
        zt_sb = big.tile([f, b_tot * npg], bf16)
        nzt = b_tot * npg // 8
        for j in range(8):
            nc.sync.dma_start(out=zt_sb[:, j * nzt:(j + 1) * nzt],
                              in_=Zt.ap()[:, j * nzt:(j + 1) * nzt])
        zl_sb = big.tile([128, rc * f], bf16)
        nc.sync.dma_start(out=zl_sb, in_=Zl.ap())

        s_bf = big.tile([128, ch_tot * k], bf16)
        if a_fp8:
            # fp8 shadow of S needed only for the LOCAL chunks (stage-1 rhs)
            s_a = big.tile([128, rc * k], adt, name="s_a")
        else:
            s_a = s_bf
        s_f32 = big.tile([128, rc * k], f32)
        njh = gpc * k // 128
        assert gpc * k == njh * 128
        ap_sb = big.tile([128, njh * b_tot * k], f32)
        zpt_sb = big.tile([f, gpc * k], f32)

        # ---- stage 0: S = softmax(Z @ W + b), 4 node-chunks per PSUM bank ----
        # The per-element has_written bit makes multiple accumulation groups
        # per bank legal: only the first matmul of a bank-round uses
        # start=True (clears the bank's bits); later matmuls overwrite
        # where unwritten / accumulate where written.
        assert ch_tot % 4 == 0

        _sstate = {}

        def softmax_tail(chk, lg_slice):
            # batch the exp-sums of 4 chunks into one tile -> 1 reciprocal op
            j = chk % 4
            if j == 0:
                _sstate["sm"] = sstat.tile([128, 4], f32, name="smb", tag="smb")
                _sstate["rs"] = sstat.tile([128, 4], f32, name="rsb", tag="rsb")
                _sstate["ex"] = []
            ex = sexp.tile([128, k], f32, name="ex", tag="ex")
            nc.scalar.activation(out=ex, in_=lg_slice, func=FT.Exp,
                                 accum_out=_sstate["sm"][:, j:j + 1])
            _sstate["ex"].append((chk, ex))
            if j == 3:
                rs = _sstate["rs"]
                nc.vector.reciprocal(rs, _sstate["sm"])
                for jj, (c2, e2) in enumerate(_sstate["ex"]):
                    nc.vector.tensor_scalar_mul(
                        s_bf[:, c2 * k:(c2 + 1) * k], e2, rs[:, jj:jj + 1])
                    if a_fp8 and c2 < rc:
                        nc.gpsimd.tensor_copy(s_a[:, c2 * k:(c2 + 1) * k],
                                              s_bf[:, c2 * k:(c2 + 1) * k])
                    if c2 < rc:
                        nc.vector.tensor_scalar_mul(
                            s_f32[:, c2 * k:(c2 + 1) * k], e2, rs[:, jj:jj + 1])

        def emit_band(bq):
            # batched: 4 node-chunks per PSUM bank, one exp per band,
            # per-chunk sums via 3D reduce, stride-0 broadcast normalize
            lg = ps_l.tile([128, 4 * k], f32, name="lg", tag="lg")
            for j in range(4):
                chk = bq * 4 + j
                nc.tensor.matmul(lg[:, j * k:(j + 1) * k],
                                 zt_sb[:, chk * 128:(chk + 1) * 128], w_sb,
                                 start=(j == 0), stop=(j == 3),
                                 skip_group_check=True)
            exb = sexp.tile([128, 4 * k], f32, name="exb", tag="exb")
            nc.scalar.activation(out=exb, in_=lg, func=FT.Exp)
            smb = sstat.tile([128, 4], f32, name="smb", tag="smb")
            nc.vector.reduce_sum(
                smb, exb.rearrange("p (j k) -> p j k", k=k),
                axis=mybir.AxisListType.X)
            rsb = sstat.tile([128, 4], f32, name="rsb", tag="rsb")
            nc.vector.reciprocal(rsb, smb)
            rs_b = bass.AP(tensor=rsb.tensor, offset=rsb.offset,
                           ap=[rsb.ap[0], rsb.ap[1], [0, k]])
            nc.vector.tensor_tensor(
                out=s_bf[:, bq * 4 * k:(bq + 1) * 4 * k].rearrange(
                    "p (j k) -> p j k", k=k),
                in0=exb.rearrange("p (j k) -> p j k", k=k),
                in1=rs_b, op=mybir.AluOpType.mult)
            if a_fp8 and bq < rc // 4:
                nc.gpsimd.tensor_copy(
                    s_a[:, bq * 4 * k:(bq + 1) * 4 * k],
                    s_bf[:, bq * 4 * k:(bq + 1) * 4 * k])
            if bq < rc // 4:
                nc.vector.tensor_tensor(
                    out=s_f32[:, bq * 4 * k:(bq + 1) * 4 * k].rearrange(
                        "p (j k) -> p j k", k=k),
                    in0=exb.rearrange("p (j k) -> p j k", k=k),
                    in1=rs_b, op=mybir.AluOpType.mult)

        nbands = ch_tot // 4
        nloc_bands = rc // 4
        if not use_bias:
            # local bands up-front; the rest interleave into the h-loop so the
            # in-order PE stream never parks blocked S-matmuls ahead of ready
            # main-loop matmuls
            for bq in range(nloc_bands):
                emit_band(bq)
        else:
            for chk in range(ch_tot):
                lg = ps_l.tile([128, k], f32)
                nc.tensor.matmul(lg, zt_sb[:, chk * 128:(chk + 1) * 128],
                                 w_sb, start=True, stop=False)
                nc.tensor.matmul(lg, ones, b_sb[:, :k], start=False,
                                 stop=True)
                softmax_tail(chk, lg)

        # ---- main loop over column blocks (positional graph index h) ----
        for h in range(b_tot):
            if not use_bias and nloc_bands + h < nbands:
                emit_band(nloc_bands + h)
            a_sb = apool.tile([128, rc * npg], adt)
            nc.sync.dma_start(out=a_sb, in_=A_t.ap()[h])
            utbs = []
            for qc in range(pcg):
                if band_ut:
                    pu = ps_ut.tile([128, gpc * k], f32)
                    first = True
                    for pc in range(pcg):
                        for g in range(gpc):
                            r = g * pcg + pc
                            nc.tensor.matmul(
                                pu[:, g * k:(g + 1) * k],
                                a_sb[:, r * npg + qc * 128:
                                     r * npg + (qc + 1) * 128],
                                s_a[:, r * k:(r + 1) * k],
                                start=first,
                                stop=(pc == pcg - 1 and g == gpc - 1),
                                skip_group_check=True)
                            first = False
                    utb = utp.tile([128, gpc * k], bf16)
                    if h >= 12 and qc == 3:
                        nc.scalar.copy(utb, pu)
                    else:
                        nc.vector.tensor_copy(utb, pu)
                else:
                    utb = utp.tile([128, gpc * k], bf16)
                    for g in range(gpc):
                        pu = ps_ut.tile([128, k], f32, name="pu_s", tag="pu_s")
                        for pc in range(pcg):
                            r = g * pcg + pc
                            nc.tensor.matmul(
                                pu,
                                a_sb[:, r * npg + qc * 128:
                                     r * npg + (qc + 1) * 128],
                                s_a[:, r * k:(r + 1) * k],
                                start=(pc == 0), stop=(pc == pcg - 1))
                        nc.vector.tensor_copy(utb[:, g * k:(g + 1) * k], pu)
                utbs.append(utb)
            # stage 2: Ap block rows = ut-halves.T @ S_h chunks (FWL-friendly
            # 128-col stationary, untransposed output)
            apbs = [ps_ap.tile([128, k], f32, name="apb", tag="apb")
                    for _ in range(njh)]
            for qc in range(pcg):
                sh = s_bf[:, (h * pcg + qc) * k:(h * pcg + qc + 1) * k]
                for j in range(njh):
                    nc.tensor.matmul(apbs[j], utbs[qc][:, j * 128:(j + 1) * 128],
                                     sh, start=(qc == 0), stop=(qc == pcg - 1))
            for j in range(njh):
                dst = ap_sb[:, j * b_tot * k + h * k: j * b_tot * k + (h + 1) * k]
                if h >= 12:
                    nc.scalar.copy(dst, apbs[j])
                else:
                    nc.vector.tensor_copy(dst, apbs[j])
            # stream finished Ap column groups out during the loop
            if h % 8 == 7:
                h0 = h - 7
                for j in range(njh):
                    nc.scalar.dma_start(
                        out=Ap_out.ap().rearrange("(j p) c -> p j c", p=128)
                        [:, j, h0 * k:(h + 1) * k],
                        in_=ap_sb[:, j * b_tot * k + h0 * k:
                                  j * b_tot * k + (h + 1) * k])

        # ---- ZpT: pooled features (transposed), local graphs ----
        if band_ut:
            zp = ps_l.tile([f, gpc * k], f32, name="zp", tag="zp", bufs=1)
            first = True
            for pc in range(pcg):
                for g in range(gpc):
                    r = g * pcg + pc
                    nc.tensor.matmul(zp[:, g * k:(g + 1) * k],
                                     zl_sb[:, r * f:(r + 1) * f],
                                     s_bf[:, r * k:(r + 1) * k],
                                     start=first,
                                     stop=(pc == pcg - 1 and g == gpc - 1),
                                     skip_group_check=True)
                    first = False
            nc.vector.tensor_copy(zpt_sb, zp)
        else:
            for g in range(gpc):
                zp = ps_l.tile([f, k], f32, name="zp_s", tag="zp_s", bufs=1)
                for pc in range(pcg):
                    r = g * pcg + pc
                    nc.tensor.matmul(zp, zl_sb[:, r * f:(r + 1) * f],
                                     s_bf[:, r * k:(r + 1) * k],
                                     start=(pc == 0), stop=(pc == pcg - 1))
                nc.vector.tensor_copy(zpt_sb[:, g * k:(g + 1) * k], zp)

        # ---- output DMAs ----
        nc.scalar.dma_start(
            out=S_out.ap().rearrange("(n p) k -> p n k", p=128),
            in_=s_f32.rearrange("p (n k) -> p n k", k=k))
        nc.scalar.dma_start(out=ZpT.ap(), in_=zpt_sb)
        if b_tot % 8 != 0:
            for j in range(njh):
                nc.scalar.dma_start(
                    out=Ap_out.ap().rearrange("(j p) c -> p j c", p=128)[:, j, :],
                    in_=ap_sb[:, j * b_tot * k:(j + 1) * b_tot * k])

    nc.compile()
    return nc


def _perm(core, b_tot=B_TOT, gpc=GPC):
    g0 = core * gpc
    return list(range(g0, g0 + gpc)) + \
        [g for g in range(b_tot) if not (g0 <= g < g0 + gpc)]


def _prep_core(Zbf, Abf, Wbf, bbf, core, b_tot=B_TOT, gpc=GPC, npg=NPG, f=F,
               k=K):
    n_loc = gpc * npg
    rc = n_loc // 128
    perm = _perm(core, b_tot, gpc)
    rows = slice(core * n_loc, (core + 1) * n_loc)

    Ar = Abf[rows].reshape(rc, 128, b_tot, npg)      # [rc, p, h_global, q]
    Ar = Ar[:, :, perm, :]                           # positional h
    A_t = np.ascontiguousarray(Ar.transpose(2, 1, 0, 3)).reshape(
        b_tot, 128, rc * npg)

    pnodes = np.concatenate([np.arange(g * npg, (g + 1) * npg) for g in perm])
    Zt = np.ascontiguousarray(Zbf[pnodes].T)         # [f, b_tot*npg]
    Zl = np.ascontiguousarray(
        Zbf[rows].reshape(rc, 128, f).transpose(1, 0, 2)).reshape(128, rc * f)
    return {"A_t": A_t, "Zt": Zt, "Zl": Zl, "Wb": Wbf,
            "bb": np.tile(bbf, 4).reshape(1, 4 * k)}


def _assemble(results, n_nodes, b_tot=B_TOT, gpc=GPC, npg=NPG, f=F, k=K,
              n_cores=N_CORES):
    n_tot = b_tot * npg
    S_local = np.ascontiguousarray(
        np.vstack([results[c]["S_out"] for c in range(n_cores)]),
        dtype=np.float32)

    Zp = np.vstack([
        results[c]["ZpT"].reshape(f, gpc, k).transpose(1, 2, 0).reshape(gpc * k, f)
        for c in range(n_cores)]).astype(np.float32, copy=False)

    ap_rows = []
    for c in range(n_cores):
        X = results[c]["Ap_out"].reshape(gpc * k, b_tot, k)  # [row, hpos, d]
        Y = np.empty_like(X)
        Y[:, _perm(c, b_tot, gpc), :] = X                    # unpermute h
        ap_rows.append(Y.reshape(gpc * k, b_tot * k))
    Ap = np.vstack(ap_rows).astype(np.float32, copy=False)

    seg_ids = np.repeat(np.arange(b_tot, dtype=np.int32), k)
    gids = np.repeat(np.arange(b_tot), n_nodes)
    Sbd = np.zeros((n_tot, b_tot * k), np.float32)
    Sbd[np.arange(n_tot)[:, None], gids[:, None] * k + np.arange(k)[None, :]] = \
        S_local
    return Zp, Ap, seg_ids, S_local, Sbd


def kernel(Z, A, n_nodes, W, b):
    from concourse.bass_utils import run_bass_kernel_spmd

    Z = np.asarray(Z, dtype=np.float32)
    A = np.asarray(A, dtype=np.float32)
    W = np.asarray(W, dtype=np.float32)
    b = np.asarray(b, dtype=np.float32)
    n_nodes = np.asarray(n_nodes)

    a_fp8 = A_FP8
    use_bias = bool(np.any(b != 0.0))
    key = (a_fp8, use_bias)
    if key not in _compiled:
        _compiled[key] = _build_nc(a_fp8=a_fp8, use_bias=use_bias)
    nc = _compiled[key]

    Zbf = Z.astype(BF16)
    Abf = A.astype(FP8 if a_fp8 else BF16)
    Wbf = W.astype(BF16)
    bbf = b.astype(BF16)
    in_maps = [_prep_core(Zbf, Abf, Wbf, bbf, c) for c in range(N_CORES)]
    res = run_bass_kernel_spmd(nc, in_maps, core_ids=list(range(N_CORES)))
    return _assemble(res.results, n_nodes)
